# revision 1
# baseline (speedup 1.0000x reference)
"""Distributed causal multi-head attention for Trainium2 (8 NeuronCores).

Problem: B=8, S=1024, D=768, H=12, DH=64 causal MHA (dense_transformer).
Sharding: pure data parallel — batch element b runs on core b; weights are
replicated. No collectives.

Per-core kernel (bf16 TensorE compute, f32 PSUM accumulation):
  1. f32->bf16 cast DMAs into DRAM staging, then xbar-transpose DMAs build
     m-major layouts: xT [m,s], wqT/wkT/wvT [m,n], woT [(head-pair h), m].
  2. QKV projections on TensorE -> QT/KT [n,s] (transposed) and V [s,n].
  3. Scores computed transposed per head: SC[q,p] = sum_h K[q,h] Q[p,h],
     two heads of a pair row-packed in the PE array (K=64 each).
     exp(SC/8) on ScalarE evicts PSUM->SBUF bf16; causal mask applied by
     gpsimd affine_select (fill 0 post-exp; fully-masked blocks skipped).
  4. z^T = V^T E accumulated on TensorE with a ones-column per head riding
     the same matmul to produce softmax denominators; normalize with
     VectorE reciprocal + gpsimd partition_broadcast + VectorE multiply.
  5. Output projection from zT/woT tiles; f32 eviction; DMA to out.
"""
import numpy as np

import concourse.bacc as bacc
import concourse.mybir as mybir
import concourse.tile as tile
from concourse.bass_utils import run_bass_kernel_spmd

f32 = mybir.dt.float32
bf16 = mybir.dt.bfloat16

B = 8
S, D, H, DH = 1024, 768, 12, 64
NT = 6    # n 128-tiles (head pairs)
MT = 6    # m 128-tiles
ST = 8    # s 128-tiles
PC = 2    # p chunks of 512
SCALE = 0.125  # 1/sqrt(DH)
W65 = DH + 1   # per-head V columns incl the ones column

N_CORES = 8


def _build_body(nc, tc, x, W_O_unused=None):
    pass


def build(n_cores: int = N_CORES):
    nc = bacc.Bacc("TRN2", target_bir_lowering=False, debug=False, num_devices=n_cores)

    x = nc.dram_tensor("x", [S, D], f32, kind="ExternalInput")
    W_Q = nc.dram_tensor("W_Q", [H, DH, D], f32, kind="ExternalInput")
    W_K = nc.dram_tensor("W_K", [H, DH, D], f32, kind="ExternalInput")
    W_V = nc.dram_tensor("W_V", [H, DH, D], f32, kind="ExternalInput")
    W_O = nc.dram_tensor("W_O", [H, D, DH], f32, kind="ExternalInput")
    out = nc.dram_tensor("out", [S, D], f32, kind="ExternalOutput")

    xbf = nc.dram_tensor("xbf", [S, D], bf16)
    wqbf = nc.dram_tensor("wqbf", [D, D], bf16)   # [(i h), m]
    wkbf = nc.dram_tensor("wkbf", [D, D], bf16)
    wvbf = nc.dram_tensor("wvbf", [D, D], bf16)
    wobf = nc.dram_tensor("wobf", [D, D], bf16)   # [m, (i h)]

    with tile.TileContext(nc) as tc:
        from contextlib import ExitStack
        with ExitStack() as ctx:
            persist = ctx.enter_context(tc.tile_pool(name="persist", bufs=1))
            epool = ctx.enter_context(tc.tile_pool(name="epool", bufs=2))
            outsb_pool = ctx.enter_context(tc.tile_pool(name="outsb", bufs=2))
            small = ctx.enter_context(tc.tile_pool(name="small", bufs=4))
            ps_mm = ctx.enter_context(tc.tile_pool(name="ps_mm", bufs=2, space="PSUM"))
            ps_sc = ctx.enter_context(tc.tile_pool(name="ps_sc", bufs=2, space="PSUM"))
            ps_zt = ctx.enter_context(tc.tile_pool(name="ps_zt", bufs=2, space="PSUM"))

            # Phase 0: cast f32 -> bf16 into DRAM staging (SWDGE cast DMA)
            nc.gpsimd.dma_start(xbf.ap(), x.ap())
            nc.gpsimd.dma_start(wqbf.ap(), W_Q.ap().rearrange("i h m -> (i h) m"))
            nc.gpsimd.dma_start(wkbf.ap(), W_K.ap().rearrange("i h m -> (i h) m"))
            nc.gpsimd.dma_start(wvbf.ap(), W_V.ap().rearrange("i h m -> (i h) m"))
            nc.gpsimd.dma_start(wobf.ap(), W_O.ap().rearrange("i m h -> m i h"))

            # Phase 1: xbar transpose loads
            xT = [persist.tile([128, S], bf16, tag=f"xT{m}", name=f"xT{m}") for m in range(MT)]
            wqT = [persist.tile([128, D], bf16, tag=f"wqT{m}", name=f"wqT{m}") for m in range(MT)]
            wkT = [persist.tile([128, D], bf16, tag=f"wkT{m}", name=f"wkT{m}") for m in range(MT)]
            wvT = [persist.tile([128, D], bf16, tag=f"wvT{m}", name=f"wvT{m}") for m in range(MT)]
            woT = [persist.tile([128, D], bf16, tag=f"woT{t}", name=f"woT{t}") for t in range(NT)]
            for m in range(MT):
                nc.sync.dma_start(xT[m][:], xbf.ap()[:, m * 128:(m + 1) * 128], transpose=True)
                nc.sync.dma_start(wqT[m][:], wqbf.ap()[:, m * 128:(m + 1) * 128], transpose=True)
                nc.sync.dma_start(wkT[m][:], wkbf.ap()[:, m * 128:(m + 1) * 128], transpose=True)
                nc.sync.dma_start(wvT[m][:], wvbf.ap()[:, m * 128:(m + 1) * 128], transpose=True)
            for t in range(NT):
                nc.sync.dma_start(woT[t][:], wobf.ap()[:, t * 128:(t + 1) * 128], transpose=True)

            V_sb = [persist.tile([128, H * W65], bf16, tag=f"V{j}", name=f"V{j}") for j in range(ST)]
            QT = [persist.tile([128, S], bf16, tag=f"QT{t}", name=f"QT{t}") for t in range(NT)]
            KT = [persist.tile([128, S], bf16, tag=f"KT{t}", name=f"KT{t}") for t in range(NT)]
            ZT = [persist.tile([128, S], bf16, tag=f"ZT{t}", name=f"ZT{t}") for t in range(NT)]

            for j in range(ST):
                ones_view = V_sb[j][:].rearrange("p (i w) -> p i w", w=W65)[:, :, DH:W65]
                nc.gpsimd.memset(ones_view, 1.0)

            def emit_v_tile(j):
                for c2 in range(2):  # n chunks of 384
                    pv = ps_mm.tile([128, 512], f32, tag="mm", name="mm")
                    for m in range(MT):
                        nc.tensor.matmul(
                            pv[:, 0:384],
                            xT[m][:, j * 128:(j + 1) * 128],
                            wvT[m][:, c2 * 384:(c2 + 1) * 384],
                            start=(m == 0), stop=(m == MT - 1),
                        )
                    dst = V_sb[j][:].rearrange("p (i w) -> p i w", w=W65)[:, c2 * 6:(c2 + 1) * 6, 0:DH]
                    src = pv[:, 0:384].rearrange("p (i w) -> p i w", w=DH)
                    nc.vector.tensor_copy(dst, src)

            def emit_qkt(t):
                for dstT, wT in ((QT, wqT), (KT, wkT)):
                    for c in range(PC):
                        pq = ps_mm.tile([128, 512], f32, tag="mm", name="mm")
                        for m in range(MT):
                            nc.tensor.matmul(
                                pq[:],
                                wT[m][:, t * 128:(t + 1) * 128],
                                xT[m][:, c * 512:(c + 1) * 512],
                                start=(m == 0), stop=(m == MT - 1),
                            )
                        nc.vector.tensor_copy(dstT[t][:, c * 512:(c + 1) * 512], pq[:])

            def emit_scores(t, E_t):
                for j in range(ST):
                    for y in range(2):
                        hb = 64 * y
                        sc = ps_sc.tile([128, 1024], f32, tag="sc", name="sc")
                        lhsT = KT[t][hb:hb + 64, j * 128:(j + 1) * 128]
                        if j <= 3:
                            nc.tensor.matmul(sc[:, 0:512], lhsT,
                                             QT[t][hb:hb + 64, 0:512],
                                             start=True, stop=True)
                        nc.tensor.matmul(sc[:, 512:1024], lhsT,
                                         QT[t][hb:hb + 64, 512:1024],
                                         start=True, stop=True)
                        if j <= 3:
                            nc.scalar.activation(
                                E_t[j][:, y * 1024:(y + 1) * 1024], sc[:],
                                mybir.ActivationFunctionType.Exp, scale=SCALE)
                            jp, dcol = j, y * 1024          # diagonal inside c0
                        else:
                            nc.scalar.activation(
                                E_t[j][:, y * 1024 + 512:(y + 1) * 1024], sc[:, 512:1024],
                                mybir.ActivationFunctionType.Exp, scale=SCALE)
                            jp, dcol = j - 4, y * 1024 + 512  # diagonal inside c1
                        dslice = E_t[j][:, dcol:dcol + 512]
                        nc.gpsimd.affine_select(
                            out=dslice, in_=dslice,
                            compare_op=mybir.AluOpType.is_ge,
                            fill=0.0, base=-128 * jp,
                            pattern=[[1, 512]], channel_multiplier=-1,
                        )

            def emit_z(t, E_t):
                for c in range(PC):
                    jmax = 4 * c + 3
                    for y in range(2):
                        i = 2 * t + y
                        zt = ps_zt.tile([128, 512], f32, tag="zt", name="zt")
                        for j in range(jmax + 1):
                            nc.tensor.matmul(
                                zt[0:65, :],
                                V_sb[j][:, i * W65:(i + 1) * W65],
                                E_t[j][:, y * 1024 + c * 512: y * 1024 + (c + 1) * 512],
                                start=(j == 0), stop=(j == jmax),
                            )
                        recip = small.tile([1, 512], f32, tag="recip", name="recip")
                        nc.vector.reciprocal(recip[:], zt[64:65, :])
                        bc = small.tile([64, 512], f32, tag="bc", name="bc")
                        nc.gpsimd.partition_broadcast(bc[:], recip[:])
                        nc.vector.tensor_mul(
                            ZT[t][64 * y:64 * y + 64, c * 512:(c + 1) * 512],
                            zt[0:64, :], bc[:])

            for j in range(4):
                emit_v_tile(j)
            emit_qkt(0)
            E_tiles = {}
            for t in range(NT):
                E_tiles[t] = [epool.tile([128, 2048], bf16, tag=f"E{j}", name=f"E{j}")
                              for j in range(ST)]
                emit_scores(t, E_tiles[t])
                if t == 0:
                    for j in range(4, ST):
                        emit_v_tile(j)
                if t + 1 < NT:
                    emit_qkt(t + 1)
                emit_z(t, E_tiles[t])

            for qj in range(ST):
                osb = outsb_pool.tile([128, D], f32, tag="osb", name="osb")
                for mc in range(2):
                    po = ps_mm.tile([128, 512], f32, tag="mm", name="mm")
                    for t in range(NT):
                        nc.tensor.matmul(
                            po[:, 0:384],
                            ZT[t][:, qj * 128:(qj + 1) * 128],
                            woT[t][:, mc * 384:(mc + 1) * 384],
                            start=(t == 0), stop=(t == NT - 1),
                        )
                    nc.scalar.copy(osb[:, mc * 384:(mc + 1) * 384], po[:, 0:384])
                nc.sync.dma_start(out.ap()[qj * 128:(qj + 1) * 128, :], osb[:])

    nc.compile()
    return nc


_NC_CACHE = None


def _get_nc():
    global _NC_CACHE
    if _NC_CACHE is None:
        _NC_CACHE = build(N_CORES)
    return _NC_CACHE


def run(inputs, trace=False, **kwargs):
    nc = _get_nc()
    weights = {k: np.ascontiguousarray(np.asarray(inputs[k], dtype=np.float32))
               for k in ("W_Q", "W_K", "W_V", "W_O")}
    xs = np.ascontiguousarray(np.asarray(inputs["x"], dtype=np.float32))
    in_maps = [dict(weights, x=xs[b]) for b in range(B)]
    res = run_bass_kernel_spmd(nc, in_maps, core_ids=list(range(N_CORES)),
                               trace=trace, **kwargs)
    out = np.stack([np.asarray(res.results[b]["out"]) for b in range(B)], axis=0)
    return out.astype(np.float32), res


def kernel(**inputs) -> np.ndarray:
    out, _ = run(inputs, trace=False)
    return out
